# revision 14
# baseline (speedup 1.0000x reference)
"""Multi-head attention (B=2, S=2048, H=16, D=64) on 8 Trainium2 NeuronCores.

Sharding: head-parallel tensor parallelism. Core c owns heads {2c, 2c+1}
(a 128-dim slice of the model dim): column-parallel QKV projections,
local causal attention for its 2 heads, row-parallel output projection
producing a partial [4096, 1024] result, then an on-device ReduceScatter
sums the partials and leaves each core with a disjoint 512-token slice of
the final output, which the host concatenates.

Matmul operands are stored as float32r (fp32 bits, TF32-like multiply at
full PE rate; ~1e-4 relative error measured). Softmax skips the
max-subtraction (logits here are O(1); exp cannot overflow) and
normalization is applied to the per-head context via the
appended-ones-column colsum trick.
"""

import sys

sys.path.insert(0, "/opt/trn_rl_repo")

import numpy as np

import concourse.bass as bass
import concourse.tile as tile
from concourse import bacc, mybir
from concourse.bass_utils import run_bass_kernel_spmd

N_CORES = 8
B, S, H, D = 2, 2048, 16, 64
E = H * D            # 1024
T = B * S            # 4096 tokens
DPC = 128            # dims (2 heads) per core
NKC = E // 128       # 8 contraction chunks for the projections
NTT = T // 512       # 8 token tiles of 512
NTB = T // 128       # 32 token blocks of 128
SB = S // 128        # 16 key blocks per batch

F32 = mybir.dt.float32
F32R = mybir.dt.float32r
AFT = mybir.ActivationFunctionType


def build_program():
    nc = bacc.Bacc("TRN2", target_bir_lowering=False, debug=False,
                   num_devices=N_CORES)

    xT = nc.dram_tensor("xT", [E, T], F32, kind="ExternalInput").ap()
    wqT = nc.dram_tensor("wqT", [E, DPC], F32, kind="ExternalInput").ap()
    wkT = nc.dram_tensor("wkT", [E, DPC], F32, kind="ExternalInput").ap()
    wvT = nc.dram_tensor("wvT", [E, DPC], F32, kind="ExternalInput").ap()
    woT = nc.dram_tensor("woT", [DPC, E], F32, kind="ExternalInput").ap()
    bq = nc.dram_tensor("bq", [DPC, 1], F32, kind="ExternalInput").ap()
    bk = nc.dram_tensor("bk", [DPC, 1], F32, kind="ExternalInput").ap()
    bv = nc.dram_tensor("bv", [DPC, 1], F32, kind="ExternalInput").ap()
    bo = nc.dram_tensor("bo", [E], F32, kind="ExternalInput").ap()
    masks = nc.dram_tensor("masks", [128, 4, 512], F32, kind="ExternalInput").ap()
    ident = nc.dram_tensor("ident", [128, 128], F32, kind="ExternalInput").ap()
    out = nc.dram_tensor("out", [T // N_CORES, E], F32, kind="ExternalOutput").ap()

    with tile.TileContext(nc) as tc:
        with (
            tc.tile_pool(name="consts", bufs=1) as consts,
            tc.tile_pool(name="state", bufs=1) as state,
            tc.tile_pool(name="xp", bufs=2) as xp,
            tc.tile_pool(name="ep", bufs=3) as ep,
            tc.tile_pool(name="cn", bufs=3) as cnp,
            tc.tile_pool(name="rp", bufs=4) as rp,
            tc.tile_pool(name="op", bufs=2) as op,
            tc.tile_pool(name="ps_a", bufs=2, space="PSUM") as ps_a,
            tc.tile_pool(name="ps_t", bufs=2, space="PSUM") as ps_t,
            tc.tile_pool(name="ps_c", bufs=4, space="PSUM") as ps_c,
            tc.tile_pool(name="dram", bufs=1, space="DRAM") as dram,
        ):
            # ---- constants -------------------------------------------------
            wq_sb = consts.tile([128, NKC, DPC], F32R)
            wk_sb = consts.tile([128, NKC, DPC], F32R)
            wv_sb = consts.tile([128, NKC, DPC], F32R)
            for kc in range(NKC):
                sl = slice(kc * 128, (kc + 1) * 128)
                nc.sync.dma_start(out=wq_sb[:, kc, :], in_=wqT[sl, :].bitcast(F32R))
                nc.sync.dma_start(out=wk_sb[:, kc, :], in_=wkT[sl, :].bitcast(F32R))
                nc.sync.dma_start(out=wv_sb[:, kc, :], in_=wvT[sl, :].bitcast(F32R))
            wo_sb = consts.tile([128, E], F32R)
            nc.sync.dma_start(out=wo_sb[:], in_=woT[:].bitcast(F32R))
            bq_sb = consts.tile([128, 1], F32)
            bk_sb = consts.tile([128, 1], F32)
            bv_sb = consts.tile([128, 1], F32)
            nc.sync.dma_start(out=bq_sb[:], in_=bq[:])
            nc.sync.dma_start(out=bk_sb[:], in_=bk[:])
            nc.sync.dma_start(out=bv_sb[:], in_=bv[:])
            bo_bc = consts.tile([128, E], F32)
            nc.sync.dma_start(
                out=bo_bc[:],
                in_=bass.AP(tensor=bo.tensor, offset=bo.offset,
                            ap=[[0, 128], [1, E]]),
            )
            mask_sb = consts.tile([128, 4, 512], F32R)
            nc.sync.dma_start(out=mask_sb[:], in_=masks[:].bitcast(F32R))
            id_sb = consts.tile([128, 128], F32)
            nc.sync.dma_start(out=id_sb[:], in_=ident[:])
            # [1, 0] per token block: the ones column (colsum) + zero pad
            # that rounds the fp32r AV matmul free dim up to an even 66.
            ones_c = consts.tile([128, NTB, 2], F32)
            nc.vector.memset(ones_c[:, :, 0:1], 1.0)
            nc.vector.memset(ones_c[:, :, 1:2], 0.0)

            # ---- persistent activations -----------------------------------
            qT_sb = state.tile([128, T], F32R)   # [2-head dims, tokens]
            kT_sb = state.tile([128, T], F32R)
            vT_sb = state.tile([128, T], F32)
            vN_sb = state.tile([128, NTB, 132], F32R)  # [tok, v dims + ones + pad]
            ctxT_sb = state.tile([128, T], F32R)  # normalized ctx, [dims, tok]

            # ones+pad columns of vN (the colsum rows of the AV matmuls)
            nc.scalar.activation(vN_sb[:, :, 64:66], ones_c[:], AFT.Copy)
            nc.scalar.activation(vN_sb[:, :, 130:132], ones_c[:], AFT.Copy)

            # ---- stage A: QKV projections (output layout [dims, tokens]) --
            for tt in range(NTT):
                x_sb = xp.tile([128, NKC, 512], F32R)
                for kc in range(NKC):
                    nc.sync.dma_start(
                        out=x_sb[:, kc, :],
                        in_=xT[kc * 128:(kc + 1) * 128,
                               tt * 512:(tt + 1) * 512].bitcast(F32R),
                    )
                for w_sb, b_sb, dst in ((wq_sb, bq_sb, qT_sb),
                                        (wk_sb, bk_sb, kT_sb),
                                        (wv_sb, bv_sb, vT_sb)):
                    ps = ps_a.tile([128, 512], F32)
                    for kc in range(NKC):
                        nc.tensor.matmul(ps[:], w_sb[:, kc, :], x_sb[:, kc, :],
                                         start=(kc == 0), stop=(kc == NKC - 1))
                    nc.vector.tensor_scalar_add(
                        dst[:, tt * 512:(tt + 1) * 512], ps[:], b_sb[:])

            # ---- stage A2: vN = v transposed to [tokens, dims] ------------
            for tb in range(NTB):
                tp_ps = ps_t.tile([128, 128], F32)
                nc.tensor.transpose(tp_ps[:], vT_sb[:, tb * 128:(tb + 1) * 128],
                                    id_sb[:])
                nc.scalar.activation(vN_sb[:, tb, 0:64], tp_ps[:, 0:64], AFT.Copy)
                nc.scalar.activation(vN_sb[:, tb, 66:130], tp_ps[:, 64:128],
                                     AFT.Copy)

            # ---- stage B: causal attention per (batch, head) --------------
            for b in range(B):
                for h in range(2):
                    d0 = h * 64
                    t0 = b * S
                    for qt in range(4):  # 512-token query tiles within batch
                        q0 = t0 + qt * 512
                        cn_tiles = [ps_c.tile([128, 66], F32, tag="cn_ps",
                                              name="cn_ps")
                                    for _ in range(4)]
                        for kb in range(4 * qt + 4):
                            s_ps = ps_a.tile([128, 512], F32, tag="ps")
                            nc.tensor.matmul(
                                s_ps[:],
                                kT_sb[d0:d0 + 64,
                                      t0 + kb * 128:t0 + (kb + 1) * 128],
                                qT_sb[d0:d0 + 64, q0:q0 + 512],
                                start=True, stop=True)
                            e_sb = ep.tile([128, 512], F32R)
                            nc.scalar.activation(e_sb[:], s_ps[:], AFT.Exp,
                                                 scale=0.125)
                            m = kb - 4 * qt
                            if m >= 0:  # diagonal tile: apply causal mask
                                nc.vector.tensor_mul(e_sb[:], e_sb[:],
                                                     mask_sb[:, m, :])
                            for qb2 in range(4):
                                qb = qt * 4 + qb2
                                if kb <= qb:
                                    nc.tensor.matmul(
                                        cn_tiles[qb2][:],
                                        e_sb[:, qb2 * 128:(qb2 + 1) * 128],
                                        vN_sb[:, b * SB + kb,
                                              h * 66:(h + 1) * 66],
                                        start=(kb == 0), stop=(kb == qb))
                        for qb2 in range(4):
                            qb = qt * 4 + qb2
                            recip = rp.tile([128, 1], F32)
                            nc.vector.reciprocal(recip[:], cn_tiles[qb2][:, 64:65])
                            cn_sb = cnp.tile([128, 64], F32)
                            nc.vector.tensor_scalar_mul(
                                cn_sb[:], cn_tiles[qb2][:, 0:64], recip[:])
                            tp2 = ps_t.tile([128, 128], F32, tag="tp_ps")
                            nc.tensor.transpose(tp2[0:64, :], cn_sb[:], id_sb[:])
                            nc.scalar.activation(
                                ctxT_sb[d0:d0 + 64,
                                        t0 + qb * 128:t0 + (qb + 1) * 128],
                                tp2[0:64, :], AFT.Copy)

            # ---- stage C: output projection + bias ------------------------
            partial = dram.tile([T, E], F32)
            for tb in range(NTB):
                o_sb = op.tile([128, E], F32)
                for et in range(2):
                    ps = ps_a.tile([128, 512], F32, tag="ps")
                    nc.tensor.matmul(ps[:],
                                     ctxT_sb[:, tb * 128:(tb + 1) * 128],
                                     wo_sb[:, et * 512:(et + 1) * 512],
                                     start=True, stop=True)
                    nc.vector.tensor_add(o_sb[:, et * 512:(et + 1) * 512], ps[:],
                                         bo_bc[:, et * 512:(et + 1) * 512])
                nc.sync.dma_start(out=partial[tb * 128:(tb + 1) * 128, :],
                                  in_=o_sb[:])

            # ---- stage D: ReduceScatter across the 8 cores ----------------
            rs_out = dram.tile([T // N_CORES, E], F32)
            nc.gpsimd.collective_compute(
                "ReduceScatter",
                mybir.AluOpType.add,
                replica_groups=[list(range(N_CORES))],
                ins=[partial.opt()],
                outs=[rs_out.opt()],
            )
            nc.sync.dma_start(out=out[:], in_=rs_out[:])

    nc.compile()
    return nc


_NC = None


def _get_program():
    global _NC
    if _NC is None:
        _NC = build_program()
    return _NC


def _causal_masks():
    m = np.zeros((128, 4, 512), np.float32)
    i = np.arange(128)[:, None]
    j = np.arange(512)[None, :]
    for s in range(4):
        m[:, s, :] = (i <= j - 128 * s).astype(np.float32)
    return m


def kernel(x, Wq, bq, Wk, bk, Wv, bv, Wo, bo, _trace=False, _trace_kwargs=None):
    x = np.asarray(x, np.float32)
    Wq, Wk, Wv, Wo = (np.asarray(w, np.float32) for w in (Wq, Wk, Wv, Wo))
    bq, bk, bv, bo = (np.asarray(v, np.float32) for v in (bq, bk, bv, bo))

    xT = np.ascontiguousarray(x.reshape(T, E).T)
    masks = _causal_masks()
    ident = np.eye(128, dtype=np.float32)
    zeros_e = np.zeros(E, np.float32)

    in_maps = []
    for c in range(N_CORES):
        sl = slice(c * DPC, (c + 1) * DPC)
        in_maps.append({
            "xT": xT,
            "wqT": np.ascontiguousarray(Wq[sl, :].T),
            "wkT": np.ascontiguousarray(Wk[sl, :].T),
            "wvT": np.ascontiguousarray(Wv[sl, :].T),
            "woT": np.ascontiguousarray(Wo[:, sl].T),
            "bq": bq[sl].reshape(DPC, 1).copy(),
            "bk": bk[sl].reshape(DPC, 1).copy(),
            "bv": bv[sl].reshape(DPC, 1).copy(),
            "bo": bo if c == 0 else zeros_e,
            "masks": masks,
            "ident": ident,
        })

    nc = _get_program()
    res = run_bass_kernel_spmd(nc, in_maps, list(range(N_CORES)),
                               trace=_trace, **(_trace_kwargs or {}))
    full = np.concatenate([res.results[i]["out"] for i in range(N_CORES)], axis=0)
    if _trace:
        return full.reshape(B, S, E), res
    return full.reshape(B, S, E)


# revision 16
# speedup vs baseline: 1.5032x; 1.5032x over previous
"""Multi-head attention (B=2, S=2048, H=16, D=64) on 8 Trainium2 NeuronCores.

Sharding: head-parallel tensor parallelism. Core c owns heads {2c, 2c+1}
(a 128-dim slice of the model dim): column-parallel QKV projections,
local causal attention for its 2 heads, row-parallel output projection
producing partial outputs, and chunked ReduceScatters (one per 512-token
group, pipelined behind compute) that sum the partials and leave each
core disjoint 64-row slices of the final output for the host to
reassemble.

Matmul operands are bf16 (fp32 PSUM accumulation everywhere; softmax
statistics and the output partial sums stay fp32) — measured end-to-end
relative error ~4e-3. Softmax skips the max-subtraction (logits here are
O(1); exp cannot overflow) and per-head normalization uses an appended
ones-column in the V operand whose matmul row yields the softmax
denominator.
"""

import sys

sys.path.insert(0, "/opt/trn_rl_repo")

import ml_dtypes
import numpy as np

import concourse.bass as bass
import concourse.tile as tile
from concourse import bacc, mybir
from concourse.bass_utils import run_bass_kernel_spmd

N_CORES = 8
B, S, H, D = 2, 2048, 16, 64
E = H * D            # 1024
T = B * S            # 4096 tokens
DPC = 128            # dims (2 heads) per core
NKC = E // 128       # 8 contraction chunks for the projections
NTT = T // 512       # 8 token tiles of 512
NTB = T // 128       # 32 token blocks of 128
SB = S // 128        # 16 key blocks per batch
NCH = 8              # reduce-scatter chunks (one per 512-token group)

F32 = mybir.dt.float32
BF16 = mybir.dt.bfloat16
AFT = mybir.ActivationFunctionType


def build_program():
    nc = bacc.Bacc("TRN2", target_bir_lowering=False, debug=False,
                   num_devices=N_CORES)

    xT = nc.dram_tensor("xT", [E, T], BF16, kind="ExternalInput").ap()
    wqT = nc.dram_tensor("wqT", [E, DPC], BF16, kind="ExternalInput").ap()
    wkT = nc.dram_tensor("wkT", [E, DPC], BF16, kind="ExternalInput").ap()
    wvT = nc.dram_tensor("wvT", [E, DPC], BF16, kind="ExternalInput").ap()
    woT = nc.dram_tensor("woT", [DPC, E], BF16, kind="ExternalInput").ap()
    bq = nc.dram_tensor("bq", [DPC, 1], F32, kind="ExternalInput").ap()
    bk = nc.dram_tensor("bk", [DPC, 1], F32, kind="ExternalInput").ap()
    bv = nc.dram_tensor("bv", [DPC, 1], F32, kind="ExternalInput").ap()
    bo = nc.dram_tensor("bo", [E], F32, kind="ExternalInput").ap()
    masks = nc.dram_tensor("masks", [128, 4, 512], BF16, kind="ExternalInput").ap()
    ident = nc.dram_tensor("ident", [128, 128], BF16, kind="ExternalInput").ap()
    out = nc.dram_tensor("out", [T // N_CORES, E], F32, kind="ExternalOutput").ap()

    with tile.TileContext(nc) as tc:
        with (
            tc.tile_pool(name="consts", bufs=1) as consts,
            tc.tile_pool(name="state", bufs=1) as state,
            tc.tile_pool(name="xp", bufs=2) as xp,
            tc.tile_pool(name="ep", bufs=3) as ep,
            tc.tile_pool(name="cn", bufs=3) as cnp,
            tc.tile_pool(name="rp", bufs=4) as rp,
            tc.tile_pool(name="op", bufs=2) as op,
            tc.tile_pool(name="ps_a", bufs=2, space="PSUM") as ps_a,
            tc.tile_pool(name="ps_t", bufs=2, space="PSUM") as ps_t,
            tc.tile_pool(name="ps_c", bufs=4, space="PSUM") as ps_c,
            tc.tile_pool(name="dram", bufs=1, space="DRAM") as dram,
        ):
            # ---- constants -------------------------------------------------
            wq_sb = consts.tile([128, NKC, DPC], BF16)
            wk_sb = consts.tile([128, NKC, DPC], BF16)
            wv_sb = consts.tile([128, NKC, DPC], BF16)
            for kc in range(NKC):
                sl = slice(kc * 128, (kc + 1) * 128)
                nc.sync.dma_start(out=wq_sb[:, kc, :], in_=wqT[sl, :])
                nc.sync.dma_start(out=wk_sb[:, kc, :], in_=wkT[sl, :])
                nc.sync.dma_start(out=wv_sb[:, kc, :], in_=wvT[sl, :])
            wo_sb = consts.tile([128, E], BF16)
            nc.sync.dma_start(out=wo_sb[:], in_=woT[:])
            bq_sb = consts.tile([128, 1], F32)
            bk_sb = consts.tile([128, 1], F32)
            bv_sb = consts.tile([128, 1], F32)
            nc.sync.dma_start(out=bq_sb[:], in_=bq[:])
            nc.sync.dma_start(out=bk_sb[:], in_=bk[:])
            nc.sync.dma_start(out=bv_sb[:], in_=bv[:])
            bo_bc = consts.tile([128, E], F32)
            nc.sync.dma_start(
                out=bo_bc[:],
                in_=bass.AP(tensor=bo.tensor, offset=bo.offset,
                            ap=[[0, 128], [1, E]]),
            )
            mask_sb = consts.tile([128, 4, 512], BF16)
            nc.sync.dma_start(out=mask_sb[:], in_=masks[:])
            id_sb = consts.tile([128, 128], BF16)
            nc.sync.dma_start(out=id_sb[:], in_=ident[:])
            # [1, 0] per token block: the ones column (softmax denominator)
            # plus a zero pad column rounding the AV free dim up to 66.
            ones_c = consts.tile([128, NTB, 2], F32)
            nc.vector.memset(ones_c[:, :, 0:1], 1.0)
            nc.vector.memset(ones_c[:, :, 1:2], 0.0)

            # ---- persistent activations -----------------------------------
            qT_sb = state.tile([128, T], BF16)   # [2-head dims, tokens]
            kT_sb = state.tile([128, T], BF16)
            vT_sb = state.tile([128, T], BF16)
            vN_sb = state.tile([128, NTB, 132], BF16)  # [tok, v dims+ones+pad]
            ctxT_sb = state.tile([128, T], BF16)  # normalized ctx, [dims, tok]

            nc.scalar.activation(vN_sb[:, :, 64:66], ones_c[:], AFT.Copy)
            nc.scalar.activation(vN_sb[:, :, 130:132], ones_c[:], AFT.Copy)

            # ---- stage A: QKV projections (output layout [dims, tokens]) --
            for tt in range(NTT):
                x_sb = xp.tile([128, NKC, 512], BF16)
                for kc in range(NKC):
                    nc.sync.dma_start(
                        out=x_sb[:, kc, :],
                        in_=xT[kc * 128:(kc + 1) * 128, tt * 512:(tt + 1) * 512],
                    )
                for w_sb, b_sb, dst in ((wq_sb, bq_sb, qT_sb),
                                        (wk_sb, bk_sb, kT_sb),
                                        (wv_sb, bv_sb, vT_sb)):
                    ps = ps_a.tile([128, 512], F32, tag="ps", name="ps")
                    for kc in range(NKC):
                        nc.tensor.matmul(ps[:], w_sb[:, kc, :], x_sb[:, kc, :],
                                         start=(kc == 0), stop=(kc == NKC - 1))
                    nc.vector.tensor_scalar_add(
                        dst[:, tt * 512:(tt + 1) * 512], ps[:], b_sb[:])

            # ---- stage A2: vN = v transposed to [tokens, dims] ------------
            for tb in range(NTB):
                tp_ps = ps_t.tile([128, 128], BF16, tag="tp_ps", name="tp_ps")
                nc.tensor.transpose(tp_ps[:], vT_sb[:, tb * 128:(tb + 1) * 128],
                                    id_sb[:])
                nc.scalar.activation(vN_sb[:, tb, 0:64], tp_ps[:, 0:64], AFT.Copy)
                nc.scalar.activation(vN_sb[:, tb, 66:130], tp_ps[:, 64:128],
                                     AFT.Copy)

            # ---- fused attention + output projection + reduce-scatter -----
            rs_outs = []
            for b in range(B):
                t0 = b * S
                for qt in range(4):  # 512-token query group within batch
                    q0 = t0 + qt * 512
                    for h in range(2):
                        d0 = h * 64
                        cn_tiles = [ps_c.tile([128, 66], F32, tag="cn_ps",
                                              name="cn_ps")
                                    for _ in range(4)]
                        for kb in range(4 * qt + 4):
                            s_ps = ps_a.tile([128, 512], F32, tag="ps",
                                             name="s_ps")
                            nc.tensor.matmul(
                                s_ps[:],
                                kT_sb[d0:d0 + 64,
                                      t0 + kb * 128:t0 + (kb + 1) * 128],
                                qT_sb[d0:d0 + 64, q0:q0 + 512],
                                start=True, stop=True)
                            e_sb = ep.tile([128, 512], BF16, tag="e_sb",
                                           name="e_sb")
                            nc.scalar.activation(e_sb[:], s_ps[:], AFT.Exp,
                                                 scale=0.125)
                            m = kb - 4 * qt
                            if m >= 0:  # diagonal tile: apply causal mask
                                nc.vector.tensor_mul(e_sb[:], e_sb[:],
                                                     mask_sb[:, m, :])
                            for qb2 in range(4):
                                qb = qt * 4 + qb2
                                if kb <= qb:
                                    nc.tensor.matmul(
                                        cn_tiles[qb2][:],
                                        e_sb[:, qb2 * 128:(qb2 + 1) * 128],
                                        vN_sb[:, b * SB + kb,
                                              h * 66:(h + 1) * 66],
                                        start=(kb == 0), stop=(kb == qb))
                        for qb2 in range(4):
                            qb = qt * 4 + qb2
                            recip = rp.tile([128, 1], F32, tag="recip",
                                            name="recip")
                            nc.vector.reciprocal(recip[:],
                                                 cn_tiles[qb2][:, 64:65])
                            cn_sb = cnp.tile([128, 64], BF16, tag="cn_sb",
                                             name="cn_sb")
                            nc.vector.tensor_scalar_mul(
                                cn_sb[:], cn_tiles[qb2][:, 0:64], recip[:])
                            tp2 = ps_t.tile([128, 128], BF16, tag="tp_ps",
                                            name="tp2")
                            nc.tensor.transpose(tp2[0:64, :], cn_sb[:], id_sb[:])
                            nc.scalar.activation(
                                ctxT_sb[d0:d0 + 64,
                                        t0 + qb * 128:t0 + (qb + 1) * 128],
                                tp2[0:64, :], AFT.Copy)

                    # output projection for this 512-token group
                    chunk = b * 4 + qt
                    partial = dram.tile([512, E], F32, tag="partial",
                                        name="partial", bufs=NCH)
                    for qb2 in range(4):
                        tb = b * SB + qt * 4 + qb2
                        o_sb = op.tile([128, E], F32, tag="o_sb", name="o_sb")
                        for et in range(2):
                            ps = ps_a.tile([128, 512], F32, tag="ps", name="c_ps")
                            nc.tensor.matmul(ps[:],
                                             ctxT_sb[:, tb * 128:(tb + 1) * 128],
                                             wo_sb[:, et * 512:(et + 1) * 512],
                                             start=True, stop=True)
                            nc.vector.tensor_add(
                                o_sb[:, et * 512:(et + 1) * 512], ps[:],
                                bo_bc[:, et * 512:(et + 1) * 512])
                        nc.sync.dma_start(
                            out=partial[qb2 * 128:(qb2 + 1) * 128, :],
                            in_=o_sb[:])

                    rs_out = dram.tile([512 // N_CORES, E], F32, tag="rs_out",
                                       name="rs_out", bufs=NCH)
                    nc.gpsimd.collective_compute(
                        "ReduceScatter",
                        mybir.AluOpType.add,
                        replica_groups=[list(range(N_CORES))],
                        ins=[partial.opt()],
                        outs=[rs_out.opt()],
                    )
                    nc.sync.dma_start(
                        out=out[chunk * 64:(chunk + 1) * 64, :],
                        in_=rs_out[:])
                    rs_outs.append(rs_out)

    nc.compile()
    return nc


_NC = None


def _get_program():
    global _NC
    if _NC is None:
        _NC = build_program()
    return _NC


def _causal_masks():
    m = np.zeros((128, 4, 512), np.float32)
    i = np.arange(128)[:, None]
    j = np.arange(512)[None, :]
    for s in range(4):
        m[:, s, :] = (i <= j - 128 * s).astype(np.float32)
    return m


def _bf(a):
    return np.ascontiguousarray(a).astype(ml_dtypes.bfloat16)


def kernel(x, Wq, bq, Wk, bk, Wv, bv, Wo, bo, _trace=False, _trace_kwargs=None):
    x = np.asarray(x, np.float32)
    Wq, Wk, Wv, Wo = (np.asarray(w, np.float32) for w in (Wq, Wk, Wv, Wo))
    bq, bk, bv, bo = (np.asarray(v, np.float32) for v in (bq, bk, bv, bo))

    xT = _bf(x.reshape(T, E).T)
    masks = _bf(_causal_masks())
    ident = _bf(np.eye(128, dtype=np.float32))
    zeros_e = np.zeros(E, np.float32)

    in_maps = []
    for c in range(N_CORES):
        sl = slice(c * DPC, (c + 1) * DPC)
        in_maps.append({
            "xT": xT,
            "wqT": _bf(Wq[sl, :].T),
            "wkT": _bf(Wk[sl, :].T),
            "wvT": _bf(Wv[sl, :].T),
            "woT": _bf(Wo[:, sl].T),
            "bq": bq[sl].reshape(DPC, 1).copy(),
            "bk": bk[sl].reshape(DPC, 1).copy(),
            "bv": bv[sl].reshape(DPC, 1).copy(),
            "bo": bo if c == 0 else zeros_e,
            "masks": masks,
            "ident": ident,
        })

    nc = _get_program()
    res = run_bass_kernel_spmd(nc, in_maps, list(range(N_CORES)),
                               trace=_trace, **(_trace_kwargs or {}))
    # out[c] rows are [chunk, 64]: chunk i holds global tokens
    # 512*i + 64*c + j.  Reassemble [NCH, N_CORES, 64, E] -> [T, E].
    stacked = np.stack([res.results[i]["out"].reshape(NCH, 64, E)
                        for i in range(N_CORES)], axis=1)
    full = stacked.reshape(T, E)
    if _trace:
        return full.reshape(B, S, E), res
    return full.reshape(B, S, E)
